# revision 5
# baseline (speedup 1.0000x reference)
"""Trainium2 Bass kernel for the CustomRNN problem.

Strategy (data-parallel over batch, 8 cores, B_local=8):
  x_{t+1} = A x_t + dd_t is a *linear* recurrence, so per core we split
  T=2048 into 4 sequential quarters; within a quarter, 16 time-chunks of 32
  steps run as one batched recurrence over 16*8=128 columns (pass A from
  zero state), chunk boundaries are fixed with a log-scan over precomputed
  powers of A, then pass B re-runs the chunks from the correct start states
  writing every state.  The drive dd and both MLP heads are large batched
  GEMMs.  Host side only does weight-derived precomputation (transposes,
  matrix powers, bias folding) and input/output relayout.
"""
import sys
import numpy as np

for _p in ('/opt/trn_rl_repo',):
    if _p not in sys.path:
        sys.path.insert(0, _p)

X, YDIM, UDIM, YH, ZH = 256, 64, 32, 64, 16
B, T = 64, 2048
NCORES = 8
BL = B // NCORES            # batch per core
NQ = 4                      # sequential quarters
TQ = T // NQ                # 512 steps / quarter
NC = 64                     # chunks per quarter
C = TQ // NC                # 8 steps per chunk
NSCAN = 6                   # log2(NC) combine-scan steps
COLS = NC * C * BL          # 4096 state columns per quarter

_CACHE = {}


def _build_program():
    if 'nc' in _CACHE:
        return _CACHE['nc'], _CACHE['names']
    from contextlib import ExitStack
    import concourse.bacc as bacc
    import concourse.tile as tile
    from concourse import mybir

    f32 = mybir.dt.float32
    nc = bacc.Bacc("TRN2", target_bir_lowering=False, debug=False,
                   num_devices=NCORES)

    # ---------------- DRAM parameters ----------------
    yT_d = nc.dram_tensor("yT", [128, NQ, 2048], f32, kind="ExternalInput")
    uT_d = nc.dram_tensor("uT", [64, NQ, 2048], f32, kind="ExternalInput")
    AwT_d = nc.dram_tensor("AwT", [X, X], f32, kind="ExternalInput")
    P_d = [nc.dram_tensor(f"P{s}T", [X, X], f32, kind="ExternalInput")
           for s in range(NSCAN)]
    KwT_d = nc.dram_tensor("KwT", [128, X], f32, kind="ExternalInput")
    BmT_d = nc.dram_tensor("BmT", [64, X], f32, kind="ExternalInput")
    Cy1T_d = nc.dram_tensor("Cy1T", [X, X], f32, kind="ExternalInput")
    Cy2T_d = nc.dram_tensor("Cy2T", [X, YH], f32, kind="ExternalInput")
    Cz1T_d = nc.dram_tensor("Cz1T", [X, X], f32, kind="ExternalInput")
    Cz2T_d = nc.dram_tensor("Cz2T", [X, ZH], f32, kind="ExternalInput")
    ddb_d = nc.dram_tensor("ddb", [128, 2], f32, kind="ExternalInput")
    cy1b_d = nc.dram_tensor("cy1b", [128, 2], f32, kind="ExternalInput")
    cz1b_d = nc.dram_tensor("cz1b", [128, 2], f32, kind="ExternalInput")
    cy2b_d = nc.dram_tensor("cy2b", [YH, 1], f32, kind="ExternalInput")
    cz2b_d = nc.dram_tensor("cz2b", [ZH, 1], f32, kind="ExternalInput")
    ypT_d = nc.dram_tensor("ypT", [YH, NQ, 4096], f32, kind="ExternalOutput")
    zpT_d = nc.dram_tensor("zpT", [ZH, NQ, 4096], f32, kind="ExternalOutput")

    AC, AM = mybir.ActivationFunctionType, mybir.AluOpType

    with tile.TileContext(nc) as tc, ExitStack() as ctx:
        wpool = ctx.enter_context(tc.tile_pool(name="w", bufs=1))
        dpool = ctx.enter_context(tc.tile_pool(name="d", bufs=2))
        xpool = ctx.enter_context(tc.tile_pool(name="x", bufs=1))
        zpool = ctx.enter_context(tc.tile_pool(name="zt", bufs=3))
        spool = ctx.enter_context(tc.tile_pool(name="st", bufs=2))
        hpool = ctx.enter_context(tc.tile_pool(name="h", bufs=3))
        opool = ctx.enter_context(tc.tile_pool(name="o", bufs=3))
        psr = ctx.enter_context(tc.tile_pool(name="psr", bufs=2, space="PSUM"))
        psb = ctx.enter_context(tc.tile_pool(name="psb", bufs=2, space="PSUM"))
        psyz = ctx.enter_context(tc.tile_pool(name="psyz", bufs=2, space="PSUM"))

        # ------------- load weights (once) -------------
        def wtile(shape, src_ap, tag):
            t = wpool.tile(shape, f32, tag=tag, name=tag)
            nc.sync.dma_start(t[:], src_ap)
            return t

        AwT = [wtile([128, X], AwT_d[k * 128:(k + 1) * 128, :], f"awt{k}")
               for k in range(2)]
        Ps = [[wtile([128, X], P_d[s][k * 128:(k + 1) * 128, :], f"p{s}{k}")
               for k in range(2)] for s in range(NSCAN)]
        KwT = wtile([128, X], KwT_d[:, :], "kwt")
        BmT = wtile([64, X], BmT_d[:, :], "bmt")
        Cy1T = [wtile([128, X], Cy1T_d[k * 128:(k + 1) * 128, :], f"cy1{k}")
                for k in range(2)]
        Cz1T = [wtile([128, X], Cz1T_d[k * 128:(k + 1) * 128, :], f"cz1{k}")
                for k in range(2)]
        Cy2T = [wtile([128, YH], Cy2T_d[k * 128:(k + 1) * 128, :], f"cy2{k}")
                for k in range(2)]
        Cz2T = [wtile([128, ZH], Cz2T_d[k * 128:(k + 1) * 128, :], f"cz2{k}")
                for k in range(2)]
        ddb = wtile([128, 2], ddb_d[:, :], "ddb")
        cy1b = wtile([128, 2], cy1b_d[:, :], "cy1b")
        cz1b = wtile([128, 2], cz1b_d[:, :], "cz1b")
        cy2b = wtile([YH, 1], cy2b_d[:, :], "cy2b")
        cz2b = wtile([ZH, 1], cz2b_d[:, :], "cz2b")

        S = None  # cross-quarter carry state, list of 2 [128, BL] tiles

        for q in range(NQ):
            # ---------------- inputs ----------------
            yT = dpool.tile([128, 2048], f32, tag="yT", name="yT")
            nc.sync.dma_start(yT[:], yT_d[:, q, :])
            uT = dpool.tile([64, 2048], f32, tag="uT", name="uT")
            nc.sync.dma_start(uT[:], uT_d[:, q, :])

            # ---------------- dd GEMM ----------------
            # dd[mb] [128, NC, C, BL]; 512-col tiles = 2 chunks each
            dd = [dpool.tile([128, NC, C, BL], f32, tag=f"dd{mb}", name=f"dd{mb}")
                  for mb in range(2)]
            for nt in range(8):
                g = nt // 4
                ysl = yT[g * 64:g * 64 + 64,
                         (nt % 4) * 512:(nt % 4) * 512 + 512]
                usl = uT[g * 32:g * 32 + 32,
                         (nt % 4) * 512:(nt % 4) * 512 + 512]
                for mb in range(2):
                    ps = psb.tile([128, 512], f32, tag="big", name="big")
                    nc.tensor.matmul(
                        ps[:], KwT[g * 64:g * 64 + 64, mb * 128:(mb + 1) * 128],
                        ysl, start=True, stop=False)
                    nc.tensor.matmul(
                        ps[:], BmT[g * 32:g * 32 + 32, mb * 128:(mb + 1) * 128],
                        usl, start=False, stop=True)
                    nc.scalar.activation(
                        dd[mb][:, nt * 8:nt * 8 + 8, :, :], ps[:],
                        AC.Identity, bias=ddb[:, mb:mb + 1])

            # ---------------- pass A (zero-start chunk recurrences) -------
            Zc = []
            for mb in range(2):
                zt = zpool.tile([128, NC, BL], f32, tag=f"z{mb}", name=f"z{mb}")
                nc.gpsimd.tensor_copy(zt[:], dd[mb][:, :, 0, :])
                Zc.append(zt)
            for j in range(1, C):
                Zn = []
                for mb in range(2):
                    ps = psr.tile([128, NC, BL], f32, tag=f"ps{mb}", name=f"ps{mb}")
                    for kb in range(2):
                        nc.tensor.matmul(
                            ps[:], AwT[kb][:, mb * 128:(mb + 1) * 128],
                            Zc[kb][:], start=(kb == 0), stop=(kb == 1))
                    zt = zpool.tile([128, NC, BL], f32, tag=f"z{mb}", name=f"z{mb}")
                    nc.vector.tensor_add(zt[:], ps[:], dd[mb][:, :, j, :])
                    Zn.append(zt)
                Zc = Zn
            F = Zc  # chunk-final states f_c, [128, NC, BL] per mb

            # ---------------- combine ----------------
            if S is not None:
                # f_0 += A^C @ S  (quarter-start correction into chunk 0)
                for mb in range(2):
                    ps = psr.tile([128, BL], f32, tag=f"ps{mb}", name=f"ps{mb}")
                    for kb in range(2):
                        nc.tensor.matmul(
                            ps[:], Ps[0][kb][:, mb * 128:(mb + 1) * 128],
                            S[kb][:], start=(kb == 0), stop=(kb == 1))
                    nc.vector.tensor_add(F[mb][:, 0, :], F[mb][:, 0, :], ps[:])
            for s in range(NSCAN):
                sh = 2 ** s
                for mb in range(2):
                    ps = psr.tile([128, NC, BL], f32, tag=f"ps{mb}", name=f"ps{mb}")
                    for kb in range(2):
                        nc.tensor.matmul(
                            ps[:, 0:NC - sh, :],
                            Ps[s][kb][:, mb * 128:(mb + 1) * 128],
                            F[kb][:, 0:NC - sh, :],
                            start=(kb == 0), stop=(kb == 1))
                    nc.vector.tensor_add(F[mb][:, sh:NC, :],
                                         F[mb][:, sh:NC, :],
                                         ps[:, 0:NC - sh, :])

            # save carry state E_{NC-1} before F tiles get recycled
            Sn = []
            for mb in range(2):
                st = spool.tile([128, BL], f32, tag=f"s{mb}", name=f"s{mb}")
                nc.gpsimd.tensor_copy(st[:], F[mb][:, NC - 1, :])
                Sn.append(st)

            # ---------------- seed pass B starts ----------------
            Xa = [xpool.tile([128, NC, C, BL], f32, tag=f"X{mb}", name=f"X{mb}")
                  for mb in range(2)]
            for mb in range(2):
                if S is None:
                    nc.vector.memset(Xa[mb][:, 0, 0, :], 0.0)
                else:
                    nc.gpsimd.tensor_copy(Xa[mb][:, 0, 0, :], S[mb][:])
                nc.gpsimd.tensor_copy(Xa[mb][:, 1:NC, 0, :],
                                      F[mb][:, 0:NC - 1, :])
            S = Sn

            # ---------------- pass B ----------------
            for j in range(C - 1):
                for mb in range(2):
                    ps = psr.tile([128, NC, BL], f32, tag=f"ps{mb}", name=f"ps{mb}")
                    for kb in range(2):
                        nc.tensor.matmul(
                            ps[:], AwT[kb][:, mb * 128:(mb + 1) * 128],
                            Xa[kb][:, :, j, :], start=(kb == 0), stop=(kb == 1))
                    nc.vector.tensor_add(Xa[mb][:, :, j + 1, :], ps[:],
                                         dd[mb][:, :, j, :])

            # ---------------- heads ----------------
            for nt in range(8):
                xs = [Xa[kb][:, nt * 8:nt * 8 + 8, :, :] for kb in range(2)]
                hy = []
                for mb in range(2):
                    ps = psb.tile([128, 512], f32, tag="big", name="big")
                    for kb in range(2):
                        nc.tensor.matmul(ps[:],
                                         Cy1T[kb][:, mb * 128:(mb + 1) * 128],
                                         xs[kb], start=(kb == 0), stop=(kb == 1))
                    ht = hpool.tile([128, 512], f32, tag=f"hy{mb}", name=f"hy{mb}")
                    nc.vector.tensor_scalar(ht[:], ps[:],
                                            cy1b[:, mb:mb + 1], 0.0,
                                            AM.add, AM.max)
                    hy.append(ht)
                psy = psyz.tile([YH, 512], f32, tag="yz", name="psy")
                for kb in range(2):
                    nc.tensor.matmul(psy[:], Cy2T[kb][:], hy[kb][:],
                                     start=(kb == 0), stop=(kb == 1))
                ypt = opool.tile([YH, 512], f32, tag="ypt", name="ypt")
                nc.scalar.activation(ypt[:], psy[:], AC.Sigmoid,
                                     bias=cy2b[:, :])
                nc.sync.dma_start(ypT_d[:, q, nt * 512:(nt + 1) * 512], ypt[:])

                hz = []
                for mb in range(2):
                    ps = psb.tile([128, 512], f32, tag="big", name="big")
                    for kb in range(2):
                        nc.tensor.matmul(ps[:],
                                         Cz1T[kb][:, mb * 128:(mb + 1) * 128],
                                         xs[kb], start=(kb == 0), stop=(kb == 1))
                    ht = hpool.tile([128, 512], f32, tag=f"hz{mb}", name=f"hz{mb}")
                    nc.vector.tensor_scalar(ht[:], ps[:],
                                            cz1b[:, mb:mb + 1], 0.0,
                                            AM.add, AM.max)
                    hz.append(ht)
                psz = psyz.tile([ZH, 512], f32, tag="yz", name="psz")
                for kb in range(2):
                    nc.tensor.matmul(psz[:], Cz2T[kb][:], hz[kb][:],
                                     start=(kb == 0), stop=(kb == 1))
                zpt = opool.tile([ZH, 512], f32, tag="zpt", name="zpt")
                nc.scalar.activation(zpt[:], psz[:], AC.Sigmoid,
                                     bias=cz2b[:, :])
                nc.sync.dma_start(zpT_d[:, q, nt * 512:(nt + 1) * 512], zpt[:])


    nc.compile()
    _CACHE['nc'] = nc
    _CACHE['names'] = None
    return nc, None


def host_prep(A_w, A_b, K_w, K_b, Bm_w, Bm_b, Cy1_w, Cy2_w, Cz1_w, Cz2_w,
              Cy1_b, Cy2_b, Cz1_b, Cz2_b):
    f = np.float32
    w = {}
    w['AwT'] = np.ascontiguousarray(A_w.T, f)
    for s in range(NSCAN):
        w[f'P{s}T'] = np.ascontiguousarray(
            np.linalg.matrix_power(A_w.astype(np.float64),
                                   C * (2 ** s)).T.astype(f))
    w['KwT'] = np.ascontiguousarray(np.concatenate([K_w.T, K_w.T], 0), f)
    w['BmT'] = np.ascontiguousarray(np.concatenate([Bm_w.T, Bm_w.T], 0), f)
    w['Cy1T'] = np.ascontiguousarray(Cy1_w.T, f)
    w['Cy2T'] = np.ascontiguousarray(Cy2_w.T, f)
    w['Cz1T'] = np.ascontiguousarray(Cz1_w.T, f)
    w['Cz2T'] = np.ascontiguousarray(Cz2_w.T, f)
    ddbias = (A_b + K_b + Bm_b).astype(f)
    w['ddb'] = np.ascontiguousarray(ddbias.reshape(2, 128).T)
    w['cy1b'] = np.ascontiguousarray(Cy1_b.astype(f).reshape(2, 128).T)
    w['cz1b'] = np.ascontiguousarray(Cz1_b.astype(f).reshape(2, 128).T)
    w['cy2b'] = np.ascontiguousarray(Cy2_b.astype(f).reshape(YH, 1))
    w['cz2b'] = np.ascontiguousarray(Cz2_b.astype(f).reshape(ZH, 1))
    return w


def pack_core_inputs(y_core, u_core):
    yT = np.ascontiguousarray(
        y_core.reshape(BL, NQ, 2, NC // 2, C, 64).transpose(2, 5, 1, 3, 4, 0)
    ).reshape(128, NQ, 2048)
    uT = np.ascontiguousarray(
        u_core.reshape(BL, NQ, 2, NC // 2, C, 32).transpose(2, 5, 1, 3, 4, 0)
    ).reshape(64, NQ, 2048)
    return yT, uT


def unpack_core_outputs(ypT, zpT):
    y = np.ascontiguousarray(
        ypT.reshape(YH, NQ, NC, C, BL).transpose(4, 1, 2, 3, 0)
    ).reshape(BL, T, YH)
    z = np.ascontiguousarray(
        zpT.reshape(ZH, NQ, NC, C, BL).transpose(4, 1, 2, 3, 0)
    ).reshape(BL, T, ZH)
    return y, z


def kernel(y, u, A_w, A_b, K_w, K_b, Bm_w, Bm_b,
           Cy1_w, Cy1_b, Cy2_w, Cy2_b, Cz1_w, Cz1_b, Cz2_w, Cz2_b,
           _run_opts=None):
    from concourse.bass_utils import run_bass_kernel_spmd

    y = np.asarray(y, np.float32)
    u = np.asarray(u, np.float32)
    nc, _ = _build_program()
    w = host_prep(np.asarray(A_w), np.asarray(A_b), np.asarray(K_w),
                  np.asarray(K_b), np.asarray(Bm_w), np.asarray(Bm_b),
                  np.asarray(Cy1_w), np.asarray(Cy2_w), np.asarray(Cz1_w),
                  np.asarray(Cz2_w), np.asarray(Cy1_b), np.asarray(Cy2_b),
                  np.asarray(Cz1_b), np.asarray(Cz2_b))
    in_maps = []
    for core in range(NCORES):
        sl = slice(core * BL, (core + 1) * BL)
        yT, uT = pack_core_inputs(y[sl], u[sl])
        m = dict(w)
        m['yT'] = yT
        m['uT'] = uT
        in_maps.append(m)
    opts = _run_opts or {}
    res = run_bass_kernel_spmd(nc, in_maps, list(range(NCORES)), **opts)
    ys, zs = [], []
    for core in range(NCORES):
        yp, zp = unpack_core_outputs(res.results[core]['ypT'],
                                     res.results[core]['zpT'])
        ys.append(yp)
        zs.append(zp)
    out = (np.concatenate(ys, 0), np.concatenate(zs, 0))
    kernel.last_results = res
    return out


# revision 9
# speedup vs baseline: 1.0723x; 1.0723x over previous
"""Trainium2 Bass kernel for the CustomRNN problem.

Strategy (data-parallel over batch, 8 cores, B_local=8):
  x_{t+1} = A x_t + dd_t is a *linear* recurrence, so per core we split
  T=2048 into 4 sequential quarters; within a quarter, 64 time-chunks of 8
  steps run as one batched recurrence over 64*8=512 columns (pass A from
  zero state), chunk boundaries are fixed with a 6-level log-scan over
  precomputed powers of A, then pass B re-runs the chunks from the correct
  start states writing every state.  The drive dd and both MLP heads are large batched
  GEMMs.  Host side only does weight-derived precomputation (transposes,
  matrix powers, bias folding) and input/output relayout.
"""
import sys
import numpy as np

for _p in ('/opt/trn_rl_repo',):
    if _p not in sys.path:
        sys.path.insert(0, _p)

X, YDIM, UDIM, YH, ZH = 256, 64, 32, 64, 16
B, T = 64, 2048
NCORES = 8
BL = B // NCORES            # batch per core
NQ = 4                      # sequential quarters
TQ = T // NQ                # 512 steps / quarter
NC = 64                     # chunks per quarter
C = TQ // NC                # 8 steps per chunk
NSCAN = 6                   # log2(NC) combine-scan steps
COLS = NC * C * BL          # 4096 state columns per quarter

_CACHE = {}


def _build_program():
    if 'nc' in _CACHE:
        return _CACHE['nc'], _CACHE['names']
    from contextlib import ExitStack
    import concourse.bacc as bacc
    import concourse.tile as tile
    from concourse import mybir

    f32 = mybir.dt.float32
    nc = bacc.Bacc("TRN2", target_bir_lowering=False, debug=False,
                   num_devices=NCORES)

    # ---------------- DRAM parameters ----------------
    yuT_d = nc.dram_tensor("yuT", [96, NQ, 4096], f32, kind="ExternalInput")
    AwT_d = nc.dram_tensor("AwT", [X, X], f32, kind="ExternalInput")
    A2T_d = nc.dram_tensor("A2T", [X, X], f32, kind="ExternalInput")
    P_d = [nc.dram_tensor(f"P{s}T", [X, X], f32, kind="ExternalInput")
           for s in range(NSCAN)]
    KwBmT_d = nc.dram_tensor("KwBmT", [96, X], f32, kind="ExternalInput")
    Cy1T_d = nc.dram_tensor("Cy1T", [X, X], f32, kind="ExternalInput")
    Cy2T_d = nc.dram_tensor("Cy2T", [X, YH], f32, kind="ExternalInput")
    Cz1T_d = nc.dram_tensor("Cz1T", [X, X], f32, kind="ExternalInput")
    Cz2T_d = nc.dram_tensor("Cz2T", [X, ZH], f32, kind="ExternalInput")
    ddb_d = nc.dram_tensor("ddb", [128, 2], f32, kind="ExternalInput")
    cy1b_d = nc.dram_tensor("cy1b", [128, 2], f32, kind="ExternalInput")
    cz1b_d = nc.dram_tensor("cz1b", [128, 2], f32, kind="ExternalInput")
    cy2b_d = nc.dram_tensor("cy2b", [YH, 1], f32, kind="ExternalInput")
    cz2b_d = nc.dram_tensor("cz2b", [ZH, 1], f32, kind="ExternalInput")
    ypT_d = nc.dram_tensor("ypT", [YH, NQ, 4096], f32, kind="ExternalOutput")
    zpT_d = nc.dram_tensor("zpT", [ZH, NQ, 4096], f32, kind="ExternalOutput")

    AC, AM = mybir.ActivationFunctionType, mybir.AluOpType

    with tile.TileContext(nc) as tc, ExitStack() as ctx:
        wpool = ctx.enter_context(tc.tile_pool(name="w", bufs=1))
        dpool = ctx.enter_context(tc.tile_pool(name="d", bufs=2))
        xpool = ctx.enter_context(tc.tile_pool(name="x", bufs=1))
        zpool = ctx.enter_context(tc.tile_pool(name="zt", bufs=3))
        spool = ctx.enter_context(tc.tile_pool(name="st", bufs=2))
        hpool = ctx.enter_context(tc.tile_pool(name="h", bufs=3))
        opool = ctx.enter_context(tc.tile_pool(name="o", bufs=3))
        psr = ctx.enter_context(tc.tile_pool(name="psr", bufs=2, space="PSUM"))
        psb = ctx.enter_context(tc.tile_pool(name="psb", bufs=2, space="PSUM"))
        psyz = ctx.enter_context(tc.tile_pool(name="psyz", bufs=2, space="PSUM"))

        # ------------- load weights (once) -------------
        def wtile(shape, src_ap, tag):
            t = wpool.tile(shape, f32, tag=tag, name=tag)
            nc.sync.dma_start(t[:], src_ap)
            return t

        AwT = [wtile([128, X], AwT_d[k * 128:(k + 1) * 128, :], f"awt{k}")
               for k in range(2)]
        A2T = [wtile([128, X], A2T_d[k * 128:(k + 1) * 128, :], f"a2t{k}")
               for k in range(2)]
        Ps = [[wtile([128, X], P_d[s][k * 128:(k + 1) * 128, :], f"p{s}{k}")
               for k in range(2)] for s in range(NSCAN)]
        KwBmT = wtile([96, X], KwBmT_d[:, :], "kwbmt")
        Cy1T = [wtile([128, X], Cy1T_d[k * 128:(k + 1) * 128, :], f"cy1{k}")
                for k in range(2)]
        Cz1T = [wtile([128, X], Cz1T_d[k * 128:(k + 1) * 128, :], f"cz1{k}")
                for k in range(2)]
        Cy2T = [wtile([128, YH], Cy2T_d[k * 128:(k + 1) * 128, :], f"cy2{k}")
                for k in range(2)]
        Cz2T = [wtile([128, ZH], Cz2T_d[k * 128:(k + 1) * 128, :], f"cz2{k}")
                for k in range(2)]
        ddb = wtile([128, 2], ddb_d[:, :], "ddb")
        cy1b = wtile([128, 2], cy1b_d[:, :], "cy1b")
        cz1b = wtile([128, 2], cz1b_d[:, :], "cz1b")
        cy2b = wtile([YH, 1], cy2b_d[:, :], "cy2b")
        cz2b = wtile([ZH, 1], cz2b_d[:, :], "cz2b")

        S = None  # cross-quarter carry state, list of 2 [128, BL] tiles

        for q in range(NQ):
            # ---------------- inputs ----------------
            yuT = dpool.tile([96, 4096], f32, tag="yuT", name="yuT")
            nc.sync.dma_start(yuT[:], yuT_d[:, q, :])

            # ---------------- dd GEMM ----------------
            # dd[mb] [128, NC, C, BL]; one K=96 matmul per 512-col tile
            dd = [dpool.tile([128, NC, C, BL], f32, tag=f"dd{mb}", name=f"dd{mb}")
                  for mb in range(2)]
            for nt in range(8):
                for mb in range(2):
                    ps = psb.tile([128, 512], f32, tag="big", name="big")
                    nc.tensor.matmul(
                        ps[:], KwBmT[:, mb * 128:(mb + 1) * 128],
                        yuT[:, nt * 512:(nt + 1) * 512], start=True, stop=True)
                    nc.scalar.activation(
                        dd[mb][:, nt * 8:nt * 8 + 8, :, :], ps[:],
                        AC.Identity, bias=ddb[:, mb:mb + 1])

            # ---------------- e2 prep (in place over dd's odd slots) ------
            # dd[:, c, 2m+1, :] <- A @ dd[:, c, 2m, :] + dd[:, c, 2m+1, :]
            for g in range(4):
                cs = slice(g * 16, g * 16 + 16)
                for mb in range(2):
                    ps = psb.tile([128, 512], f32, tag="big", name="big")
                    for kb in range(2):
                        nc.tensor.matmul(
                            ps[:], AwT[kb][:, mb * 128:(mb + 1) * 128],
                            dd[kb][:, cs, 0:C:2, :], start=(kb == 0),
                            stop=(kb == 1))
                    nc.vector.tensor_add(dd[mb][:, cs, 1:C:2, :],
                                         dd[mb][:, cs, 1:C:2, :], ps[:])

            # ---------------- pass A (zero-start, distance-2 steps) -------
            Zc = []
            for mb in range(2):
                zt = zpool.tile([128, NC, BL], f32, tag=f"z{mb}", name=f"z{mb}")
                nc.gpsimd.tensor_copy(zt[:], dd[mb][:, :, 1, :])
                Zc.append(zt)
            for m in range(1, C // 2):
                Zn = []
                for mb in range(2):
                    ps = psr.tile([128, NC, BL], f32, tag=f"ps{mb}", name=f"ps{mb}")
                    for kb in range(2):
                        nc.tensor.matmul(
                            ps[:], A2T[kb][:, mb * 128:(mb + 1) * 128],
                            Zc[kb][:], start=(kb == 0), stop=(kb == 1))
                    zt = zpool.tile([128, NC, BL], f32, tag=f"z{mb}", name=f"z{mb}")
                    nc.vector.tensor_add(zt[:], ps[:], dd[mb][:, :, 2 * m + 1, :])
                    Zn.append(zt)
                Zc = Zn
            F = Zc  # chunk-final states f_c, [128, NC, BL] per mb

            # ---------------- combine ----------------
            if S is not None:
                # f_0 += A^C @ S  (quarter-start correction into chunk 0)
                for mb in range(2):
                    ps = psr.tile([128, BL], f32, tag=f"ps{mb}", name=f"ps{mb}")
                    for kb in range(2):
                        nc.tensor.matmul(
                            ps[:], Ps[0][kb][:, mb * 128:(mb + 1) * 128],
                            S[kb][:], start=(kb == 0), stop=(kb == 1))
                    nc.vector.tensor_add(F[mb][:, 0, :], F[mb][:, 0, :], ps[:])
            for s in range(NSCAN):
                sh = 2 ** s
                for mb in range(2):
                    ps = psr.tile([128, NC, BL], f32, tag=f"ps{mb}", name=f"ps{mb}")
                    for kb in range(2):
                        nc.tensor.matmul(
                            ps[:, 0:NC - sh, :],
                            Ps[s][kb][:, mb * 128:(mb + 1) * 128],
                            F[kb][:, 0:NC - sh, :],
                            start=(kb == 0), stop=(kb == 1))
                    nc.vector.tensor_add(F[mb][:, sh:NC, :],
                                         F[mb][:, sh:NC, :],
                                         ps[:, 0:NC - sh, :])

            # save carry state E_{NC-1} before F tiles get recycled
            Sn = []
            for mb in range(2):
                st = spool.tile([128, BL], f32, tag=f"s{mb}", name=f"s{mb}")
                nc.gpsimd.tensor_copy(st[:], F[mb][:, NC - 1, :])
                Sn.append(st)

            # ---------------- seed pass B starts ----------------
            Xa = [xpool.tile([128, NC, C, BL], f32, tag=f"X{mb}", name=f"X{mb}")
                  for mb in range(2)]
            for mb in range(2):
                if S is None:
                    nc.vector.memset(Xa[mb][:, 0, 0, :], 0.0)
                else:
                    nc.gpsimd.tensor_copy(Xa[mb][:, 0, 0, :], S[mb][:])
                nc.gpsimd.tensor_copy(Xa[mb][:, 1:NC, 0, :],
                                      F[mb][:, 0:NC - 1, :])
            S = Sn

            # ---------------- pass B (even states, distance-2) ----------
            for m in range(C // 2 - 1):
                for mb in range(2):
                    ps = psr.tile([128, NC, BL], f32, tag=f"ps{mb}", name=f"ps{mb}")
                    for kb in range(2):
                        nc.tensor.matmul(
                            ps[:], A2T[kb][:, mb * 128:(mb + 1) * 128],
                            Xa[kb][:, :, 2 * m, :], start=(kb == 0),
                            stop=(kb == 1))
                    nc.vector.tensor_add(Xa[mb][:, :, 2 * m + 2, :], ps[:],
                                         dd[mb][:, :, 2 * m + 1, :])

            # ---------------- odd-state recovery (batched GEMM) ----------
            # x_{2m+1} = A x_{2m} + dd_{2m}
            for g in range(4):
                cs = slice(g * 16, g * 16 + 16)
                for mb in range(2):
                    ps = psb.tile([128, 512], f32, tag="big", name="big")
                    for kb in range(2):
                        nc.tensor.matmul(
                            ps[:], AwT[kb][:, mb * 128:(mb + 1) * 128],
                            Xa[kb][:, cs, 0:C:2, :], start=(kb == 0),
                            stop=(kb == 1))
                    nc.vector.tensor_add(Xa[mb][:, cs, 1:C:2, :], ps[:],
                                         dd[mb][:, cs, 0:C:2, :])

            # ---------------- heads ----------------
            for nt in range(8):
                xs = [Xa[kb][:, nt * 8:nt * 8 + 8, :, :] for kb in range(2)]
                hy = []
                for mb in range(2):
                    ps = psb.tile([128, 512], f32, tag="big", name="big")
                    for kb in range(2):
                        nc.tensor.matmul(ps[:],
                                         Cy1T[kb][:, mb * 128:(mb + 1) * 128],
                                         xs[kb], start=(kb == 0), stop=(kb == 1))
                    ht = hpool.tile([128, 512], f32, tag=f"hy{mb}", name=f"hy{mb}")
                    nc.vector.tensor_scalar(ht[:], ps[:],
                                            cy1b[:, mb:mb + 1], 0.0,
                                            AM.add, AM.max)
                    hy.append(ht)
                psy = psyz.tile([YH, 512], f32, tag="yz", name="psy")
                for kb in range(2):
                    nc.tensor.matmul(psy[:], Cy2T[kb][:], hy[kb][:],
                                     start=(kb == 0), stop=(kb == 1))
                ypt = opool.tile([YH, 512], f32, tag="ypt", name="ypt")
                nc.scalar.activation(ypt[:], psy[:], AC.Sigmoid,
                                     bias=cy2b[:, :])
                nc.sync.dma_start(ypT_d[:, q, nt * 512:(nt + 1) * 512], ypt[:])

                hz = []
                for mb in range(2):
                    ps = psb.tile([128, 512], f32, tag="big", name="big")
                    for kb in range(2):
                        nc.tensor.matmul(ps[:],
                                         Cz1T[kb][:, mb * 128:(mb + 1) * 128],
                                         xs[kb], start=(kb == 0), stop=(kb == 1))
                    ht = hpool.tile([128, 512], f32, tag=f"hz{mb}", name=f"hz{mb}")
                    nc.vector.tensor_scalar(ht[:], ps[:],
                                            cz1b[:, mb:mb + 1], 0.0,
                                            AM.add, AM.max)
                    hz.append(ht)
                psz = psyz.tile([ZH, 512], f32, tag="yz", name="psz")
                for kb in range(2):
                    nc.tensor.matmul(psz[:], Cz2T[kb][:], hz[kb][:],
                                     start=(kb == 0), stop=(kb == 1))
                zpt = opool.tile([ZH, 512], f32, tag="zpt", name="zpt")
                nc.scalar.activation(zpt[:], psz[:], AC.Sigmoid,
                                     bias=cz2b[:, :])
                nc.sync.dma_start(zpT_d[:, q, nt * 512:(nt + 1) * 512], zpt[:])


    nc.compile()
    _CACHE['nc'] = nc
    _CACHE['names'] = None
    return nc, None


def host_prep(A_w, A_b, K_w, K_b, Bm_w, Bm_b, Cy1_w, Cy2_w, Cz1_w, Cz2_w,
              Cy1_b, Cy2_b, Cz1_b, Cz2_b):
    f = np.float32
    w = {}
    w['AwT'] = np.ascontiguousarray(A_w.T, f)
    w['A2T'] = np.ascontiguousarray((A_w @ A_w).T, f)
    for s in range(NSCAN):
        w[f'P{s}T'] = np.ascontiguousarray(
            np.linalg.matrix_power(A_w.astype(np.float64),
                                   C * (2 ** s)).T.astype(f))
    w['KwBmT'] = np.ascontiguousarray(np.concatenate([K_w.T, Bm_w.T], 0), f)
    w['Cy1T'] = np.ascontiguousarray(Cy1_w.T, f)
    w['Cy2T'] = np.ascontiguousarray(Cy2_w.T, f)
    w['Cz1T'] = np.ascontiguousarray(Cz1_w.T, f)
    w['Cz2T'] = np.ascontiguousarray(Cz2_w.T, f)
    ddbias = (A_b + K_b + Bm_b).astype(f)
    w['ddb'] = np.ascontiguousarray(ddbias.reshape(2, 128).T)
    w['cy1b'] = np.ascontiguousarray(Cy1_b.astype(f).reshape(2, 128).T)
    w['cz1b'] = np.ascontiguousarray(Cz1_b.astype(f).reshape(2, 128).T)
    w['cy2b'] = np.ascontiguousarray(Cy2_b.astype(f).reshape(YH, 1))
    w['cz2b'] = np.ascontiguousarray(Cz2_b.astype(f).reshape(ZH, 1))
    return w


def pack_core_inputs(y_core, u_core):
    yv = y_core.reshape(BL, NQ, NC, C, 64).transpose(4, 1, 2, 3, 0)\
        .reshape(64, NQ, 4096)
    uv = u_core.reshape(BL, NQ, NC, C, 32).transpose(4, 1, 2, 3, 0)\
        .reshape(32, NQ, 4096)
    return np.ascontiguousarray(np.concatenate([yv, uv], 0)),


def unpack_core_outputs(ypT, zpT):
    y = np.ascontiguousarray(
        ypT.reshape(YH, NQ, NC, C, BL).transpose(4, 1, 2, 3, 0)
    ).reshape(BL, T, YH)
    z = np.ascontiguousarray(
        zpT.reshape(ZH, NQ, NC, C, BL).transpose(4, 1, 2, 3, 0)
    ).reshape(BL, T, ZH)
    return y, z


def kernel(y, u, A_w, A_b, K_w, K_b, Bm_w, Bm_b,
           Cy1_w, Cy1_b, Cy2_w, Cy2_b, Cz1_w, Cz1_b, Cz2_w, Cz2_b,
           _run_opts=None):
    from concourse.bass_utils import run_bass_kernel_spmd

    y = np.asarray(y, np.float32)
    u = np.asarray(u, np.float32)
    nc, _ = _build_program()
    w = host_prep(np.asarray(A_w), np.asarray(A_b), np.asarray(K_w),
                  np.asarray(K_b), np.asarray(Bm_w), np.asarray(Bm_b),
                  np.asarray(Cy1_w), np.asarray(Cy2_w), np.asarray(Cz1_w),
                  np.asarray(Cz2_w), np.asarray(Cy1_b), np.asarray(Cy2_b),
                  np.asarray(Cz1_b), np.asarray(Cz2_b))
    in_maps = []
    for core in range(NCORES):
        sl = slice(core * BL, (core + 1) * BL)
        (yuT,) = pack_core_inputs(y[sl], u[sl])
        m = dict(w)
        m['yuT'] = yuT
        in_maps.append(m)
    opts = _run_opts or {}
    res = run_bass_kernel_spmd(nc, in_maps, list(range(NCORES)), **opts)
    ys, zs = [], []
    for core in range(NCORES):
        yp, zp = unpack_core_outputs(res.results[core]['ypT'],
                                     res.results[core]['zpT'])
        ys.append(yp)
        zs.append(zp)
    out = (np.concatenate(ys, 0), np.concatenate(zs, 0))
    kernel.last_results = res
    return out
